# revision 7
# baseline (speedup 1.0000x reference)
"""Trainium2 Bass kernel for nn_Memory scatter_memory problem.

Reference computation:
    scale = t/(t+1) if t > 1 else 1
    inv   = 1/(t+1)
    entity_memory = entity_memory*scale ;  .at[nodes_ids].add((nodes_emb @ W_node.T + b_node)*inv)
    rel_memory    = rel_memory*scale    ;  .at[rels_ids].add((rels_emb @ W_rel.T + b_rel)*inv)
    out = concat([entity_memory, rel_memory])   # [100500, 512]

Strategy (8 NeuronCores, SPMD single program):
  - Row-shard entity_memory (12544 rows/core) and rel_memory (64 rows/core).
  - HOST routes each event to its owner core (by id range), sorts by local row id,
    pads to a common chunk count, pre-transposes embeddings to [ev-chunk, p=kdim, 8, 128]
    layout, and pre-scales W/b by inv (so device updates are final).
  - DEVICE per core: fp32r projection matmuls (events on PSUM partitions), then
    scatter-add via one-hot matmuls into per-row-group PSUM tiles (exact fp32
    accumulation, duplicates handled by matmul), then out = mem*scale + psum.
  - Host reassembles the full [100500, 512] output from per-core shards.
"""

import os
import sys
import numpy as np

for _p in ("/root/.axon_site", "/root/.axon_site/_ro/trn_rl_repo",
           "/root/.axon_site/_ro/pypackages", "/opt/trn_rl_repo"):
    if os.path.isdir(_p) and _p not in sys.path:
        sys.path.append(_p)

import concourse.bacc as bacc
import concourse.mybir as mybir
import concourse.tile as tile
from concourse.bass_utils import run_bass_kernel_spmd

F32 = mybir.dt.float32
F32R = mybir.dt.float32r
F16 = mybir.dt.float16
AL = mybir.AluOpType

N_NODES = 100000
N_RELS = 500
MEM_DIM = 512
IN_DIM = 1024
NCORES = 8
NSHARD = 12544          # 98 * 128 rows per core (core 7 ragged, padded)
NGROUPS = NSHARD // 128  # 98
RSHARD = 64             # rel rows per core (core 7 ragged, padded)
KT = IN_DIM // 128      # 8 k-tiles
PAD_ID = 1.0e6

_module_cache = {}


def _ensure_ntff_hook():
    """Register the axon NTFF profile hook (missing antenv.axon_hooks shim)."""
    import types
    try:
        from antenv.axon_hooks import get_axon_ntff_profile_hook
        return get_axon_ntff_profile_hook() is not None
    except ImportError:
        pass
    try:
        import antenv
        from trn_agent_boot.trn_boot import _ntff_profile_via_ctypes
        import concourse.bass_utils as bu
        mod = types.ModuleType("antenv.axon_hooks")
        state = {"h": None}
        mod.set_axon_ntff_profile_hook = lambda h: state.__setitem__("h", h)
        mod.get_axon_ntff_profile_hook = lambda: state["h"]
        sys.modules["antenv.axon_hooks"] = mod
        antenv.axon_hooks = mod
        h = _ntff_profile_via_ctypes("/opt/axon/libaxon_pjrt.so")
        mod.set_axon_ntff_profile_hook(h)
        bu.upload_artifacts = lambda tmpdir: f"local:{tmpdir}"
        return h is not None
    except Exception:
        return False


def _build_module(NCn, NCr, spans_n):
    """Build the SPMD Bacc module.

    NCn/NCr: number of 128-event chunks for nodes/rels.
    spans_n: list over ec of sorted group lists (union over cores).
    """
    nc = bacc.Bacc(None, target_bir_lowering=False)

    emb_n = nc.dram_tensor("emb_n", [NCn, 128, KT * 128], F16, kind="ExternalInput")
    emb_r = nc.dram_tensor("emb_r", [NCr, 128, KT * 128], F16, kind="ExternalInput")
    ids_n = nc.dram_tensor("ids_n", [128, NCn], F32, kind="ExternalInput")
    ids_r = nc.dram_tensor("ids_r", [128, NCr], F32, kind="ExternalInput")
    w_n = nc.dram_tensor("w_n", [128, KT * MEM_DIM], F16, kind="ExternalInput")
    w_r = nc.dram_tensor("w_r", [128, KT * MEM_DIM], F16, kind="ExternalInput")
    b_n = nc.dram_tensor("b_n", [128, MEM_DIM], F32, kind="ExternalInput")
    b_r = nc.dram_tensor("b_r", [128, MEM_DIM], F32, kind="ExternalInput")
    s_col = nc.dram_tensor("s_col", [128, 1], F32, kind="ExternalInput")
    iota_in = nc.dram_tensor("iota_in", [128, 128], F32, kind="ExternalInput")
    mem = nc.dram_tensor("mem", [NSHARD, MEM_DIM], F32, kind="ExternalInput")
    rmem = nc.dram_tensor("rmem", [RSHARD, MEM_DIM], F32, kind="ExternalInput")
    out_n = nc.dram_tensor("out_n", [NSHARD, MEM_DIM], F32, kind="ExternalOutput")
    out_r = nc.dram_tensor("out_r", [RSHARD, MEM_DIM], F32, kind="ExternalOutput")

    # which chunk finishes each node group (merge point)
    last_chunk = {}
    for ec, gs in enumerate(spans_n):
        for g in gs:
            last_chunk[g] = ec
    merge_after = [[] for _ in range(NCn)]
    for g, ec in last_chunk.items():
        merge_after[ec].append(g)
    untouched = [g for g in range(NGROUPS) if g not in last_chunk]

    # PSUM budget: proj double-buffer + open scatter groups + rel accumulator
    maxopen = 0
    open_now = set()
    for ec, gs in enumerate(spans_n):
        open_now.update(gs)
        maxopen = max(maxopen, len(open_now))
        for g in merge_after[ec]:
            open_now.discard(g)
    pu_bufs = 2 if maxopen <= 5 else 1
    pg_bufs = min(max(maxopen, 1), 8 - pu_bufs - 1)

    with tile.TileContext(nc) as tc:
        with tc.tile_pool(name="const", bufs=1) as cpool, \
             tc.tile_pool(name="stage", bufs=10) as spool, \
             tc.tile_pool(name="work", bufs=14) as wpool, \
             tc.tile_pool(name="oh", bufs=16) as ohpool, \
             tc.tile_pool(name="updp", bufs=8) as updpool, \
             tc.tile_pool(name="pu", bufs=pu_bufs, space="PSUM") as pupool, \
             tc.tile_pool(name="pg", bufs=pg_bufs, space="PSUM") as pgpool, \
             tc.tile_pool(name="pr", bufs=1, space="PSUM") as prpool:

            # ---- constants ----
            t_iota = cpool.tile([128, 128], F32, tag="iota")
            nc.sync.dma_start(t_iota[:], iota_in[:])
            t_ids_n = cpool.tile([128, NCn], F32, tag="idsn")
            nc.sync.dma_start(t_ids_n[:], ids_n[:])
            t_ids_r = cpool.tile([128, NCr], F32, tag="idsr")
            nc.sync.dma_start(t_ids_r[:], ids_r[:])
            t_bn = cpool.tile([128, MEM_DIM], F32, tag="bn")
            nc.sync.dma_start(t_bn[:], b_n[:])
            t_br = cpool.tile([128, MEM_DIM], F32, tag="br")
            nc.sync.dma_start(t_br[:], b_r[:])
            t_s = cpool.tile([128, 1], F32, tag="scol")
            nc.sync.dma_start(t_s[:], s_col[:])

            t_wn = cpool.tile([128, KT, MEM_DIM], F16, tag="wn")
            nc.sync.dma_start(t_wn[:], w_n.ap().rearrange("p (k n) -> p k n", k=KT))
            t_wr = cpool.tile([128, KT, MEM_DIM], F16, tag="wr")
            nc.sync.dma_start(t_wr[:], w_r.ap().rearrange("p (k n) -> p k n", k=KT))

            def merge_group(g):
                t_mem = wpool.tile([128, MEM_DIM], F32, tag="memst")
                nc.sync.dma_start(t_mem[:], mem[g * 128:(g + 1) * 128, :])
                t_out = wpool.tile([128, MEM_DIM], F32, tag="outsb")
                if g in grp_psum:
                    nc.vector.scalar_tensor_tensor(
                        t_out[:], t_mem[:], t_s[:, 0:1], grp_psum[g][:],
                        op0=AL.mult, op1=AL.add)
                    del grp_psum[g]
                else:
                    nc.vector.tensor_scalar_mul(t_out[:], t_mem[:], t_s[:, 0:1])
                nc.sync.dma_start(out_n[g * 128:(g + 1) * 128, :], t_out[:])

            grp_psum = {}

            def node_chunk(ec):
                t_en = spool.tile([128, KT, 128], F16, tag="er", name=f"en_{ec}")
                nc.sync.dma_start(t_en[:], emb_n[ec].rearrange("p (k j) -> p k j", k=KT))
                p_u = pupool.tile([128, MEM_DIM], F32, tag="pu", name=f"pun_{ec}")
                for k in range(KT):
                    nc.tensor.matmul(p_u[:], t_en[:, k, :], t_wn[:, k, :],
                                     start=(k == 0), stop=(k == KT - 1))
                t_upd = updpool.tile([128, MEM_DIM], F32R, tag="upd", name=f"updn_{ec}")
                nc.vector.tensor_tensor(t_upd[:], p_u[:], t_bn[:], op=AL.add)
                for g in spans_n[ec]:
                    t_oh = ohpool.tile([128, 128], F32R, tag="oh", name=f"ohn_{ec}_{g}")
                    nc.vector.tensor_scalar(
                        t_oh[:], t_iota[:], float(g * 128), t_ids_n[:, ec:ec + 1],
                        op0=AL.add, op1=AL.is_equal)
                    if g not in grp_psum:
                        grp_psum[g] = pgpool.tile([128, MEM_DIM], F32, tag="pg",
                                                  name=f"pg_{g}")
                        first = True
                    else:
                        first = False
                    nc.tensor.matmul(grp_psum[g][:], t_oh[:], t_upd[:],
                                     start=first, stop=(last_chunk[g] == ec),
                                     skip_group_check=True)
                for g in sorted(merge_after[ec]):
                    merge_group(g)

            def rel_chunk(ec):
                t_er2 = spool.tile([128, KT, 128], F16, tag="er", name=f"er_{ec}")
                nc.sync.dma_start(t_er2[:], emb_r[ec].rearrange("p (k j) -> p k j", k=KT))
                p_u = pupool.tile([128, MEM_DIM], F32, tag="pu", name=f"pur_{ec}")
                for k in range(KT):
                    nc.tensor.matmul(p_u[:], t_er2[:, k, :], t_wr[:, k, :],
                                     start=(k == 0), stop=(k == KT - 1))
                t_upd = updpool.tile([128, MEM_DIM], F32R, tag="upd", name=f"updr_{ec}")
                nc.vector.tensor_tensor(t_upd[:], p_u[:], t_br[:], op=AL.add)
                t_oh = ohpool.tile([128, 128], F32R, tag="oh", name=f"ohr_{ec}")
                nc.vector.tensor_scalar(
                    t_oh[:], t_iota[:], 0.0, t_ids_r[:, ec:ec + 1],
                    op0=AL.add, op1=AL.is_equal)
                nc.tensor.matmul(p_rel[:64, :], t_oh[:, :64], t_upd[:],
                                 start=(ec == 0), stop=(ec == NCr - 1),
                                 skip_group_check=True)

            # interleave node/rel chunks to smooth engine mix
            p_rel = prpool.tile([128, MEM_DIM], F32, tag="prel")
            for i in range(max(NCn, NCr)):
                if i < NCn:
                    node_chunk(i)
                if i < NCr:
                    rel_chunk(i)

            for g in untouched:
                merge_group(g)

            # ---- rel merge ----
            t_rmem = wpool.tile([128, MEM_DIM], F32, tag="memst")
            nc.sync.dma_start(t_rmem[:64, :], rmem[:])
            t_rout = wpool.tile([128, MEM_DIM], F32, tag="outsb")
            nc.vector.scalar_tensor_tensor(
                t_rout[:64, :], t_rmem[:64, :], t_s[:64, 0:1], p_rel[:64, :],
                op0=AL.mult, op1=AL.add)
            nc.sync.dma_start(out_r[:], t_rout[:64, :])

    nc.finalize()
    return nc


def _route(ids, n_rows_per_core, pad_chunks=1):
    """Route events to owner cores; sort by local id.

    Returns (perm[core] event indices sorted by local id, NC common chunk count).
    """
    owner = np.minimum(ids // n_rows_per_core, NCORES - 1)
    perms = []
    for c in range(NCORES):
        ev = np.nonzero(owner == c)[0]
        loc = ids[ev] - c * n_rows_per_core
        order = np.argsort(loc, kind="stable")
        perms.append(ev[order])
    nmax = max(len(p) for p in perms)
    NC = (nmax + 127) // 128
    return perms, max(NC, 1)


def _rnd_f32r(x):
    b = x.view(np.uint32)
    low = b & np.uint32(0xFFF)
    keep = b & ~np.uint32(0xFFF)
    rup = keep + np.uint32(0x1000)
    use_up = (low > 0x800) | ((low == 0x800) & (((b >> 12) & 1) == 1))
    return np.where(use_up, rup, keep).view(np.float32)


def _pack_emb(embT, perm, NC):
    """embT [IN_DIM, B] fp32 -> [NC, 128, KT*128] routed/padded/pretiled."""
    n = len(perm)
    C = NC * 128
    # gather columns -> [IN_DIM, C]
    g = np.zeros((IN_DIM, C), dtype=embT.dtype)
    g[:, :n] = embT[:, perm]
    # [KT,128,NC,128] -> [NC, p=128(kdim), KT, 128(event)]
    g = g.reshape(KT, 128, NC, 128).transpose(2, 1, 0, 3).reshape(NC, 128, KT * 128)
    return np.ascontiguousarray(g)


def _pack_ids(local_ids, NC):
    n = len(local_ids)
    C = NC * 128
    out = np.full(C, PAD_ID, dtype=np.float32)
    out[:n] = local_ids.astype(np.float32)
    return np.ascontiguousarray(out.reshape(NC, 128).T)  # [128, NC]


def _spans(local_sorted_per_core, NC):
    spans = [set() for _ in range(NC)]
    for loc in local_sorted_per_core:
        for ec in range(NC):
            seg = loc[ec * 128:(ec + 1) * 128]
            if len(seg) == 0:
                continue
            for g in range(int(seg[0]) // 128, int(seg[-1]) // 128 + 1):
                spans[ec].add(g)
    return [sorted(s) for s in spans]


def kernel(nodes_embeddings, rels_embeddings, nodes_ids, rels_ids,
           entity_memory, rel_memory, W_node, b_node, W_rel, b_rel, time):
    nodes_embeddings = np.ascontiguousarray(np.asarray(nodes_embeddings, dtype=np.float32))
    rels_embeddings = np.ascontiguousarray(np.asarray(rels_embeddings, dtype=np.float32))
    nodes_ids = np.asarray(nodes_ids).astype(np.int64)
    rels_ids = np.asarray(rels_ids).astype(np.int64)
    entity_memory = np.asarray(entity_memory, dtype=np.float32)
    rel_memory = np.asarray(rel_memory, dtype=np.float32)
    W_node = np.asarray(W_node, dtype=np.float32)
    b_node = np.asarray(b_node, dtype=np.float32)
    W_rel = np.asarray(W_rel, dtype=np.float32)
    b_rel = np.asarray(b_rel, dtype=np.float32)
    t = float(np.asarray(time))

    inv = np.float32(1.0 / (t + 1.0))
    scale = np.float32(t / (t + 1.0)) if t > 1 else np.float32(1.0)

    # ---- host routing ----
    perms_n, NCn = _route(nodes_ids, NSHARD)
    perms_r, NCr = _route(rels_ids, RSHARD)

    loc_n = [nodes_ids[p] - c * NSHARD for c, p in enumerate(perms_n)]
    spans_n = _spans(loc_n, NCn)

    key = (NCn, NCr, tuple(tuple(s) for s in spans_n))
    if key not in _module_cache:
        _module_cache[key] = _build_module(NCn, NCr, spans_n)
    nc = _module_cache[key]

    # ---- host packing ----
    embT_n = nodes_embeddings.astype(np.float16).T  # [IN_DIM, B]
    embT_r = rels_embeddings.astype(np.float16).T
    wn = np.ascontiguousarray(
        (W_node * inv).T.reshape(KT, 128, MEM_DIM).transpose(1, 0, 2)
        .reshape(128, KT * MEM_DIM)).astype(np.float16)
    wr = np.ascontiguousarray(
        (W_rel * inv).T.reshape(KT, 128, MEM_DIM).transpose(1, 0, 2)
        .reshape(128, KT * MEM_DIM)).astype(np.float16)
    bn = np.broadcast_to(b_node * inv, (128, MEM_DIM)).astype(np.float32).copy()
    br = np.broadcast_to(b_rel * inv, (128, MEM_DIM)).astype(np.float32).copy()
    s_col = np.full((128, 1), scale, dtype=np.float32)
    iota = np.broadcast_to(np.arange(128, dtype=np.float32), (128, 128)).copy()

    in_maps = []
    for c in range(NCORES):
        lo_n, hi_n = c * NSHARD, min((c + 1) * NSHARD, N_NODES)
        lo_r, hi_r = c * RSHARD, min((c + 1) * RSHARD, N_RELS)
        mem_shard = np.zeros((NSHARD, MEM_DIM), dtype=np.float32)
        mem_shard[:hi_n - lo_n] = entity_memory[lo_n:hi_n]
        rmem_shard = np.zeros((RSHARD, MEM_DIM), dtype=np.float32)
        rmem_shard[:hi_r - lo_r] = rel_memory[lo_r:hi_r]
        in_maps.append(dict(
            emb_n=_pack_emb(embT_n, perms_n[c], NCn),
            emb_r=_pack_emb(embT_r, perms_r[c], NCr),
            ids_n=_pack_ids(loc_n[c], NCn),
            ids_r=_pack_ids(rels_ids[perms_r[c]] - c * RSHARD, NCr),
            w_n=wn, w_r=wr, b_n=bn, b_r=br, s_col=s_col, iota_in=iota,
            mem=mem_shard, rmem=rmem_shard,
        ))

    trace = bool(int(os.environ.get("KERNEL_TRACE", "0"))) and _ensure_ntff_hook()
    res = run_bass_kernel_spmd(
        nc, in_maps, core_ids=list(range(NCORES)),
        trace=trace, trace_cores=list(range(NCORES)) if trace else None)
    kernel.last_exec_time_ns = res.exec_time_ns
    kernel.last_results = res

    out = np.empty((N_NODES + N_RELS, MEM_DIM), dtype=np.float32)
    for c in range(NCORES):
        lo_n, hi_n = c * NSHARD, min((c + 1) * NSHARD, N_NODES)
        out[lo_n:hi_n] = res.results[c]["out_n"][:hi_n - lo_n]
        lo_r, hi_r = c * RSHARD, min((c + 1) * RSHARD, N_RELS)
        out[N_NODES + lo_r:N_NODES + hi_r] = res.results[c]["out_r"][:hi_r - lo_r]
    return out


# revision 8
# speedup vs baseline: 1.2165x; 1.2165x over previous
"""Trainium2 Bass kernel for nn_Memory scatter_memory problem.

Reference computation:
    scale = t/(t+1) if t > 1 else 1
    inv   = 1/(t+1)
    entity_memory = entity_memory*scale ;  .at[nodes_ids].add((nodes_emb @ W_node.T + b_node)*inv)
    rel_memory    = rel_memory*scale    ;  .at[rels_ids].add((rels_emb @ W_rel.T + b_rel)*inv)
    out = concat([entity_memory, rel_memory])   # [100500, 512]

Strategy (8 NeuronCores, SPMD single program):
  - Row-shard entity_memory (12544 rows/core) and rel_memory (64 rows/core).
  - HOST routes each event to its owner core (by id range), sorts by local row id,
    pads to a common chunk count, pre-transposes embeddings to [ev-chunk, p=kdim, 8, 128]
    layout, and pre-scales W/b by inv (so device updates are final).
  - DEVICE per core: fp32r projection matmuls (events on PSUM partitions), then
    scatter-add via one-hot matmuls into per-row-group PSUM tiles (exact fp32
    accumulation, duplicates handled by matmul), then out = mem*scale + psum.
  - Host reassembles the full [100500, 512] output from per-core shards.
"""

import os
import sys
import numpy as np

for _p in ("/root/.axon_site", "/root/.axon_site/_ro/trn_rl_repo",
           "/root/.axon_site/_ro/pypackages", "/opt/trn_rl_repo"):
    if os.path.isdir(_p) and _p not in sys.path:
        sys.path.append(_p)

import concourse.bacc as bacc
import concourse.mybir as mybir
import concourse.tile as tile
from concourse.bass_utils import run_bass_kernel_spmd

F32 = mybir.dt.float32
F32R = mybir.dt.float32r
F16 = mybir.dt.float16
AL = mybir.AluOpType

N_NODES = 100000
N_RELS = 500
MEM_DIM = 512
IN_DIM = 1024
NCORES = 8
NSHARD = 12544          # 98 * 128 rows per core (core 7 ragged, padded)
NGROUPS = NSHARD // 128  # 98
RSHARD = 64             # rel rows per core (core 7 ragged, padded)
KT = IN_DIM // 128      # 8 k-tiles
PAD_ID = 1.0e6

_module_cache = {}


def _ensure_ntff_hook():
    """Register the axon NTFF profile hook (missing antenv.axon_hooks shim)."""
    import types
    try:
        from antenv.axon_hooks import get_axon_ntff_profile_hook
        return get_axon_ntff_profile_hook() is not None
    except ImportError:
        pass
    try:
        import antenv
        from trn_agent_boot.trn_boot import _ntff_profile_via_ctypes
        import concourse.bass_utils as bu
        mod = types.ModuleType("antenv.axon_hooks")
        state = {"h": None}
        mod.set_axon_ntff_profile_hook = lambda h: state.__setitem__("h", h)
        mod.get_axon_ntff_profile_hook = lambda: state["h"]
        sys.modules["antenv.axon_hooks"] = mod
        antenv.axon_hooks = mod
        h = _ntff_profile_via_ctypes("/opt/axon/libaxon_pjrt.so")
        mod.set_axon_ntff_profile_hook(h)
        bu.upload_artifacts = lambda tmpdir: f"local:{tmpdir}"
        return h is not None
    except Exception:
        return False


def _build_module(NCn, NCr, spans_n):
    """Build the SPMD Bacc module.

    NCn/NCr: number of 128-event chunks for nodes/rels.
    spans_n: list over ec of sorted group lists (union over cores).
    """
    nc = bacc.Bacc(None, target_bir_lowering=False)

    emb_n = nc.dram_tensor("emb_n", [NCn, 128, KT * 128], F16, kind="ExternalInput")
    emb_r = nc.dram_tensor("emb_r", [NCr, 128, KT * 128], F16, kind="ExternalInput")
    ids_n = nc.dram_tensor("ids_n", [128, NCn], F32, kind="ExternalInput")
    ids_r = nc.dram_tensor("ids_r", [128, NCr], F32, kind="ExternalInput")
    w_n = nc.dram_tensor("w_n", [128, KT * MEM_DIM], F16, kind="ExternalInput")
    w_r = nc.dram_tensor("w_r", [128, KT * MEM_DIM], F16, kind="ExternalInput")
    b_n = nc.dram_tensor("b_n", [128, MEM_DIM], F32, kind="ExternalInput")
    b_r = nc.dram_tensor("b_r", [128, MEM_DIM], F32, kind="ExternalInput")
    s_col = nc.dram_tensor("s_col", [128, 1], F32, kind="ExternalInput")
    iota_in = nc.dram_tensor("iota_in", [128, 128], F32, kind="ExternalInput")
    mem = nc.dram_tensor("mem", [NSHARD, MEM_DIM], F32, kind="ExternalInput")
    rmem = nc.dram_tensor("rmem", [RSHARD, MEM_DIM], F32, kind="ExternalInput")
    out_n = nc.dram_tensor("out_n", [NSHARD, MEM_DIM], F32, kind="ExternalOutput")
    out_r = nc.dram_tensor("out_r", [RSHARD, MEM_DIM], F32, kind="ExternalOutput")

    # which chunk finishes each node group (merge point)
    last_chunk = {}
    for ec, gs in enumerate(spans_n):
        for g in gs:
            last_chunk[g] = ec
    merge_after = [[] for _ in range(NCn)]
    for g, ec in last_chunk.items():
        merge_after[ec].append(g)
    untouched = [g for g in range(NGROUPS) if g not in last_chunk]

    # PSUM budget: proj double-buffer + open scatter groups + rel accumulator
    maxopen = 0
    open_now = set()
    for ec, gs in enumerate(spans_n):
        open_now.update(gs)
        maxopen = max(maxopen, len(open_now))
        for g in merge_after[ec]:
            open_now.discard(g)
    pu_bufs = 2 if maxopen <= 5 else 1
    pg_bufs = min(max(maxopen, 1), 8 - pu_bufs - 1)

    with tile.TileContext(nc) as tc:
        with tc.tile_pool(name="const", bufs=1) as cpool, \
             tc.tile_pool(name="stage", bufs=10) as spool, \
             tc.tile_pool(name="work", bufs=14) as wpool, \
             tc.tile_pool(name="oh", bufs=16) as ohpool, \
             tc.tile_pool(name="updp", bufs=8) as updpool, \
             tc.tile_pool(name="pu", bufs=pu_bufs, space="PSUM") as pupool, \
             tc.tile_pool(name="pg", bufs=pg_bufs, space="PSUM") as pgpool, \
             tc.tile_pool(name="pr", bufs=1, space="PSUM") as prpool:

            # ---- constants ----
            t_iota = cpool.tile([128, 128], F32, tag="iota")
            nc.sync.dma_start(t_iota[:], iota_in[:])
            t_ids_n = cpool.tile([128, NCn], F32, tag="idsn")
            nc.sync.dma_start(t_ids_n[:], ids_n[:])
            t_ids_r = cpool.tile([128, NCr], F32, tag="idsr")
            nc.sync.dma_start(t_ids_r[:], ids_r[:])
            t_bn = cpool.tile([128, MEM_DIM], F32, tag="bn")
            nc.sync.dma_start(t_bn[:], b_n[:])
            t_br = cpool.tile([128, MEM_DIM], F32, tag="br")
            nc.sync.dma_start(t_br[:], b_r[:])
            t_s = cpool.tile([128, 1], F32, tag="scol")
            nc.sync.dma_start(t_s[:], s_col[:])

            t_wn = cpool.tile([128, KT, MEM_DIM], F16, tag="wn")
            nc.sync.dma_start(t_wn[:], w_n.ap().rearrange("p (k n) -> p k n", k=KT))
            t_wr = cpool.tile([128, KT, MEM_DIM], F16, tag="wr")
            nc.sync.dma_start(t_wr[:], w_r.ap().rearrange("p (k n) -> p k n", k=KT))

            def merge_group(g):
                t_mem = wpool.tile([128, MEM_DIM], F32, tag="memst")
                nc.sync.dma_start(t_mem[:], mem[g * 128:(g + 1) * 128, :])
                t_out = wpool.tile([128, MEM_DIM], F32, tag="outsb")
                if g in grp_psum:
                    nc.vector.scalar_tensor_tensor(
                        t_out[:], t_mem[:], t_s[:, 0:1], grp_psum[g][:],
                        op0=AL.mult, op1=AL.add)
                    del grp_psum[g]
                else:
                    nc.vector.tensor_scalar_mul(t_out[:], t_mem[:], t_s[:, 0:1])
                nc.sync.dma_start(out_n[g * 128:(g + 1) * 128, :], t_out[:])

            grp_psum = {}
            upd_n = {}
            upd_r = {}

            def node_proj(ec):
                t_en = spool.tile([128, KT, 128], F16, tag="er", name=f"en_{ec}")
                nc.sync.dma_start(t_en[:], emb_n[ec].rearrange("p (k j) -> p k j", k=KT))
                p_u = pupool.tile([128, MEM_DIM], F32, tag="pu", name=f"pun_{ec}")
                for k in range(KT):
                    nc.tensor.matmul(p_u[:], t_en[:, k, :], t_wn[:, k, :],
                                     start=(k == 0), stop=(k == KT - 1))
                t_upd = updpool.tile([128, MEM_DIM], F32R, tag="upd", name=f"updn_{ec}")
                nc.vector.tensor_tensor(t_upd[:], p_u[:], t_bn[:], op=AL.add)
                upd_n[ec] = t_upd

            def node_scatter(ec):
                t_upd = upd_n.pop(ec)
                for g in spans_n[ec]:
                    t_oh = ohpool.tile([128, 128], F32R, tag="oh", name=f"ohn_{ec}_{g}")
                    nc.vector.tensor_scalar(
                        t_oh[:], t_iota[:], float(g * 128), t_ids_n[:, ec:ec + 1],
                        op0=AL.add, op1=AL.is_equal)
                    if g not in grp_psum:
                        grp_psum[g] = pgpool.tile([128, MEM_DIM], F32, tag="pg",
                                                  name=f"pg_{g}")
                        first = True
                    else:
                        first = False
                    nc.tensor.matmul(grp_psum[g][:], t_oh[:], t_upd[:],
                                     start=first, stop=(last_chunk[g] == ec),
                                     skip_group_check=True)
                for g in sorted(merge_after[ec]):
                    merge_group(g)

            def rel_proj(ec):
                t_er2 = spool.tile([128, KT, 128], F16, tag="er", name=f"er_{ec}")
                nc.sync.dma_start(t_er2[:], emb_r[ec].rearrange("p (k j) -> p k j", k=KT))
                p_u = pupool.tile([128, MEM_DIM], F32, tag="pu", name=f"pur_{ec}")
                for k in range(KT):
                    nc.tensor.matmul(p_u[:], t_er2[:, k, :], t_wr[:, k, :],
                                     start=(k == 0), stop=(k == KT - 1))
                t_upd = updpool.tile([128, MEM_DIM], F32R, tag="upd", name=f"updr_{ec}")
                nc.vector.tensor_tensor(t_upd[:], p_u[:], t_br[:], op=AL.add)
                upd_r[ec] = t_upd

            def rel_scatter(ec):
                t_upd = upd_r.pop(ec)
                t_oh = ohpool.tile([128, 128], F32R, tag="oh", name=f"ohr_{ec}")
                nc.vector.tensor_scalar(
                    t_oh[:], t_iota[:], 0.0, t_ids_r[:, ec:ec + 1],
                    op0=AL.add, op1=AL.is_equal)
                nc.tensor.matmul(p_rel[:64, :], t_oh[:, :64], t_upd[:],
                                 start=(ec == 0), stop=(ec == NCr - 1),
                                 skip_group_check=True)

            # software-pipelined emission: scatter runs one chunk behind proj,
            # node/rel interleaved to smooth the engine mix
            p_rel = prpool.tile([128, MEM_DIM], F32, tag="prel")
            steps = []
            for i in range(max(NCn, NCr)):
                if i < NCn:
                    steps.append(("n", i))
                if i < NCr:
                    steps.append(("r", i))
            for j, (kind, i) in enumerate(steps):
                (node_proj if kind == "n" else rel_proj)(i)
                if j >= 1:
                    pk, pi = steps[j - 1]
                    (node_scatter if pk == "n" else rel_scatter)(pi)
            lk, li = steps[-1]
            (node_scatter if lk == "n" else rel_scatter)(li)

            for g in untouched:
                merge_group(g)

            # ---- rel merge ----
            t_rmem = wpool.tile([128, MEM_DIM], F32, tag="memst")
            nc.sync.dma_start(t_rmem[:64, :], rmem[:])
            t_rout = wpool.tile([128, MEM_DIM], F32, tag="outsb")
            nc.vector.scalar_tensor_tensor(
                t_rout[:64, :], t_rmem[:64, :], t_s[:64, 0:1], p_rel[:64, :],
                op0=AL.mult, op1=AL.add)
            nc.sync.dma_start(out_r[:], t_rout[:64, :])

    nc.finalize()
    return nc


def _route(ids, n_rows_per_core, pad_chunks=1):
    """Route events to owner cores; sort by local id.

    Returns (perm[core] event indices sorted by local id, NC common chunk count).
    """
    owner = np.minimum(ids // n_rows_per_core, NCORES - 1)
    perms = []
    for c in range(NCORES):
        ev = np.nonzero(owner == c)[0]
        loc = ids[ev] - c * n_rows_per_core
        order = np.argsort(loc, kind="stable")
        perms.append(ev[order])
    nmax = max(len(p) for p in perms)
    NC = (nmax + 127) // 128
    return perms, max(NC, 1)


def _rnd_f32r(x):
    b = x.view(np.uint32)
    low = b & np.uint32(0xFFF)
    keep = b & ~np.uint32(0xFFF)
    rup = keep + np.uint32(0x1000)
    use_up = (low > 0x800) | ((low == 0x800) & (((b >> 12) & 1) == 1))
    return np.where(use_up, rup, keep).view(np.float32)


def _pack_emb(embT, perm, NC):
    """embT [IN_DIM, B] fp32 -> [NC, 128, KT*128] routed/padded/pretiled."""
    n = len(perm)
    C = NC * 128
    # gather columns -> [IN_DIM, C]
    g = np.zeros((IN_DIM, C), dtype=embT.dtype)
    g[:, :n] = embT[:, perm]
    # [KT,128,NC,128] -> [NC, p=128(kdim), KT, 128(event)]
    g = g.reshape(KT, 128, NC, 128).transpose(2, 1, 0, 3).reshape(NC, 128, KT * 128)
    return np.ascontiguousarray(g)


def _pack_ids(local_ids, NC):
    n = len(local_ids)
    C = NC * 128
    out = np.full(C, PAD_ID, dtype=np.float32)
    out[:n] = local_ids.astype(np.float32)
    return np.ascontiguousarray(out.reshape(NC, 128).T)  # [128, NC]


def _spans(local_sorted_per_core, NC):
    spans = [set() for _ in range(NC)]
    for loc in local_sorted_per_core:
        for ec in range(NC):
            seg = loc[ec * 128:(ec + 1) * 128]
            if len(seg) == 0:
                continue
            for g in range(int(seg[0]) // 128, int(seg[-1]) // 128 + 1):
                spans[ec].add(g)
    return [sorted(s) for s in spans]


def kernel(nodes_embeddings, rels_embeddings, nodes_ids, rels_ids,
           entity_memory, rel_memory, W_node, b_node, W_rel, b_rel, time):
    nodes_embeddings = np.ascontiguousarray(np.asarray(nodes_embeddings, dtype=np.float32))
    rels_embeddings = np.ascontiguousarray(np.asarray(rels_embeddings, dtype=np.float32))
    nodes_ids = np.asarray(nodes_ids).astype(np.int64)
    rels_ids = np.asarray(rels_ids).astype(np.int64)
    entity_memory = np.asarray(entity_memory, dtype=np.float32)
    rel_memory = np.asarray(rel_memory, dtype=np.float32)
    W_node = np.asarray(W_node, dtype=np.float32)
    b_node = np.asarray(b_node, dtype=np.float32)
    W_rel = np.asarray(W_rel, dtype=np.float32)
    b_rel = np.asarray(b_rel, dtype=np.float32)
    t = float(np.asarray(time))

    inv = np.float32(1.0 / (t + 1.0))
    scale = np.float32(t / (t + 1.0)) if t > 1 else np.float32(1.0)

    # ---- host routing ----
    perms_n, NCn = _route(nodes_ids, NSHARD)
    perms_r, NCr = _route(rels_ids, RSHARD)

    loc_n = [nodes_ids[p] - c * NSHARD for c, p in enumerate(perms_n)]
    spans_n = _spans(loc_n, NCn)

    key = (NCn, NCr, tuple(tuple(s) for s in spans_n))
    if key not in _module_cache:
        _module_cache[key] = _build_module(NCn, NCr, spans_n)
    nc = _module_cache[key]

    # ---- host packing ----
    embT_n = nodes_embeddings.astype(np.float16).T  # [IN_DIM, B]
    embT_r = rels_embeddings.astype(np.float16).T
    wn = np.ascontiguousarray(
        (W_node * inv).T.reshape(KT, 128, MEM_DIM).transpose(1, 0, 2)
        .reshape(128, KT * MEM_DIM)).astype(np.float16)
    wr = np.ascontiguousarray(
        (W_rel * inv).T.reshape(KT, 128, MEM_DIM).transpose(1, 0, 2)
        .reshape(128, KT * MEM_DIM)).astype(np.float16)
    bn = np.broadcast_to(b_node * inv, (128, MEM_DIM)).astype(np.float32).copy()
    br = np.broadcast_to(b_rel * inv, (128, MEM_DIM)).astype(np.float32).copy()
    s_col = np.full((128, 1), scale, dtype=np.float32)
    iota = np.broadcast_to(np.arange(128, dtype=np.float32), (128, 128)).copy()

    in_maps = []
    for c in range(NCORES):
        lo_n, hi_n = c * NSHARD, min((c + 1) * NSHARD, N_NODES)
        lo_r, hi_r = c * RSHARD, min((c + 1) * RSHARD, N_RELS)
        mem_shard = np.zeros((NSHARD, MEM_DIM), dtype=np.float32)
        mem_shard[:hi_n - lo_n] = entity_memory[lo_n:hi_n]
        rmem_shard = np.zeros((RSHARD, MEM_DIM), dtype=np.float32)
        rmem_shard[:hi_r - lo_r] = rel_memory[lo_r:hi_r]
        in_maps.append(dict(
            emb_n=_pack_emb(embT_n, perms_n[c], NCn),
            emb_r=_pack_emb(embT_r, perms_r[c], NCr),
            ids_n=_pack_ids(loc_n[c], NCn),
            ids_r=_pack_ids(rels_ids[perms_r[c]] - c * RSHARD, NCr),
            w_n=wn, w_r=wr, b_n=bn, b_r=br, s_col=s_col, iota_in=iota,
            mem=mem_shard, rmem=rmem_shard,
        ))

    trace = bool(int(os.environ.get("KERNEL_TRACE", "0"))) and _ensure_ntff_hook()
    res = run_bass_kernel_spmd(
        nc, in_maps, core_ids=list(range(NCORES)),
        trace=trace, trace_cores=list(range(NCORES)) if trace else None)
    kernel.last_exec_time_ns = res.exec_time_ns
    kernel.last_results = res

    out = np.empty((N_NODES + N_RELS, MEM_DIM), dtype=np.float32)
    for c in range(NCORES):
        lo_n, hi_n = c * NSHARD, min((c + 1) * NSHARD, N_NODES)
        out[lo_n:hi_n] = res.results[c]["out_n"][:hi_n - lo_n]
        lo_r, hi_r = c * RSHARD, min((c + 1) * RSHARD, N_RELS)
        out[N_NODES + lo_r:N_NODES + hi_r] = res.results[c]["out_r"][:hi_r - lo_r]
    return out


# revision 9
# speedup vs baseline: 1.2171x; 1.0005x over previous
"""Trainium2 Bass kernel for nn_Memory scatter_memory problem.

Reference computation:
    scale = t/(t+1) if t > 1 else 1
    inv   = 1/(t+1)
    entity_memory = entity_memory*scale ;  .at[nodes_ids].add((nodes_emb @ W_node.T + b_node)*inv)
    rel_memory    = rel_memory*scale    ;  .at[rels_ids].add((rels_emb @ W_rel.T + b_rel)*inv)
    out = concat([entity_memory, rel_memory])   # [100500, 512]

Strategy (8 NeuronCores, SPMD single program):
  - Row-shard entity_memory (12544 rows/core) and rel_memory (64 rows/core).
  - HOST routes each event to its owner core (by id range), sorts by local row id,
    pads to a common chunk count, pre-transposes embeddings to [ev-chunk, p=kdim, 8, 128]
    layout, and pre-scales W/b by inv (so device updates are final).
  - DEVICE per core: fp32r projection matmuls (events on PSUM partitions), then
    scatter-add via one-hot matmuls into per-row-group PSUM tiles (exact fp32
    accumulation, duplicates handled by matmul), then out = mem*scale + psum.
  - Host reassembles the full [100500, 512] output from per-core shards.
"""

import os
import sys
import numpy as np

for _p in ("/root/.axon_site", "/root/.axon_site/_ro/trn_rl_repo",
           "/root/.axon_site/_ro/pypackages", "/opt/trn_rl_repo"):
    if os.path.isdir(_p) and _p not in sys.path:
        sys.path.append(_p)

import concourse.bacc as bacc
import concourse.mybir as mybir
import concourse.tile as tile
from concourse.bass_utils import run_bass_kernel_spmd

F32 = mybir.dt.float32
F32R = mybir.dt.float32r
F16 = mybir.dt.float16
AL = mybir.AluOpType

N_NODES = 100000
N_RELS = 500
MEM_DIM = 512
IN_DIM = 1024
NCORES = 8
NSHARD = 12544          # 98 * 128 rows per core (core 7 ragged, padded)
NGROUPS = NSHARD // 128  # 98
RSHARD = 64             # rel rows per core (core 7 ragged, padded)
KT = IN_DIM // 128      # 8 k-tiles
PAD_ID = 1.0e6

_module_cache = {}


def _ensure_ntff_hook():
    """Register the axon NTFF profile hook (missing antenv.axon_hooks shim)."""
    import types
    try:
        from antenv.axon_hooks import get_axon_ntff_profile_hook
        return get_axon_ntff_profile_hook() is not None
    except ImportError:
        pass
    try:
        import antenv
        from trn_agent_boot.trn_boot import _ntff_profile_via_ctypes
        import concourse.bass_utils as bu
        mod = types.ModuleType("antenv.axon_hooks")
        state = {"h": None}
        mod.set_axon_ntff_profile_hook = lambda h: state.__setitem__("h", h)
        mod.get_axon_ntff_profile_hook = lambda: state["h"]
        sys.modules["antenv.axon_hooks"] = mod
        antenv.axon_hooks = mod
        h = _ntff_profile_via_ctypes("/opt/axon/libaxon_pjrt.so")
        mod.set_axon_ntff_profile_hook(h)
        bu.upload_artifacts = lambda tmpdir: f"local:{tmpdir}"
        return h is not None
    except Exception:
        return False


def _build_module(NCn, NCr, spans_n):
    """Build the SPMD Bacc module.

    NCn/NCr: number of 128-event chunks for nodes/rels.
    spans_n: list over ec of sorted group lists (union over cores).
    """
    nc = bacc.Bacc(None, target_bir_lowering=False)

    emb_n = nc.dram_tensor("emb_n", [NCn, 128, KT * 128], F16, kind="ExternalInput")
    emb_r = nc.dram_tensor("emb_r", [NCr, 128, KT * 128], F16, kind="ExternalInput")
    ids_n = nc.dram_tensor("ids_n", [128, NCn], F32, kind="ExternalInput")
    ids_r = nc.dram_tensor("ids_r", [128, NCr], F32, kind="ExternalInput")
    w_n = nc.dram_tensor("w_n", [128, KT * MEM_DIM], F16, kind="ExternalInput")
    w_r = nc.dram_tensor("w_r", [128, KT * MEM_DIM], F16, kind="ExternalInput")
    b_n = nc.dram_tensor("b_n", [128, MEM_DIM], F32, kind="ExternalInput")
    b_r = nc.dram_tensor("b_r", [128, MEM_DIM], F32, kind="ExternalInput")
    s_col = nc.dram_tensor("s_col", [128, 1], F32, kind="ExternalInput")
    iota_in = nc.dram_tensor("iota_in", [128, 128], F32, kind="ExternalInput")
    mem = nc.dram_tensor("mem", [NSHARD, MEM_DIM], F32, kind="ExternalInput")
    rmem = nc.dram_tensor("rmem", [RSHARD, MEM_DIM], F32, kind="ExternalInput")
    out_n = nc.dram_tensor("out_n", [NSHARD, MEM_DIM], F32, kind="ExternalOutput")
    out_r = nc.dram_tensor("out_r", [RSHARD, MEM_DIM], F32, kind="ExternalOutput")

    # which chunk finishes each node group (merge point)
    last_chunk = {}
    for ec, gs in enumerate(spans_n):
        for g in gs:
            last_chunk[g] = ec
    merge_after = [[] for _ in range(NCn)]
    for g, ec in last_chunk.items():
        merge_after[ec].append(g)
    untouched = [g for g in range(NGROUPS) if g not in last_chunk]

    # PSUM budget: proj double-buffer + open scatter groups + rel accumulator
    maxopen = 0
    open_now = set()
    for ec, gs in enumerate(spans_n):
        open_now.update(gs)
        maxopen = max(maxopen, len(open_now))
        for g in merge_after[ec]:
            open_now.discard(g)
    pu_bufs = 2 if maxopen <= 5 else 1
    pg_bufs = min(max(maxopen, 1), 8 - pu_bufs - 1)

    with tile.TileContext(nc) as tc:
        with tc.tile_pool(name="const", bufs=1) as cpool, \
             tc.tile_pool(name="stage", bufs=10) as spool, \
             tc.tile_pool(name="work", bufs=14) as wpool, \
             tc.tile_pool(name="oh", bufs=16) as ohpool, \
             tc.tile_pool(name="updp", bufs=8) as updpool, \
             tc.tile_pool(name="pu", bufs=pu_bufs, space="PSUM") as pupool, \
             tc.tile_pool(name="pg", bufs=pg_bufs, space="PSUM") as pgpool, \
             tc.tile_pool(name="pr", bufs=1, space="PSUM") as prpool:

            # ---- constants (W first: PE-critical path) ----
            t_wn = cpool.tile([128, KT, MEM_DIM], F16, tag="wn")
            nc.sync.dma_start(t_wn[:], w_n.ap().rearrange("p (k n) -> p k n", k=KT))
            t_wr = cpool.tile([128, KT, MEM_DIM], F16, tag="wr")
            nc.sync.dma_start(t_wr[:], w_r.ap().rearrange("p (k n) -> p k n", k=KT))
            t_iota = cpool.tile([128, 128], F32, tag="iota")
            nc.sync.dma_start(t_iota[:], iota_in[:])
            t_ids_n = cpool.tile([128, NCn], F32, tag="idsn")
            nc.sync.dma_start(t_ids_n[:], ids_n[:])
            t_ids_r = cpool.tile([128, NCr], F32, tag="idsr")
            nc.sync.dma_start(t_ids_r[:], ids_r[:])
            t_bn = cpool.tile([128, MEM_DIM], F32, tag="bn")
            nc.sync.dma_start(t_bn[:], b_n[:])
            t_br = cpool.tile([128, MEM_DIM], F32, tag="br")
            nc.sync.dma_start(t_br[:], b_r[:])
            t_s = cpool.tile([128, 1], F32, tag="scol")
            nc.sync.dma_start(t_s[:], s_col[:])

            def merge_group(g):
                t_mem = wpool.tile([128, MEM_DIM], F32, tag="memst")
                nc.sync.dma_start(t_mem[:], mem[g * 128:(g + 1) * 128, :])
                t_out = wpool.tile([128, MEM_DIM], F32, tag="outsb")
                if g in grp_psum:
                    nc.vector.scalar_tensor_tensor(
                        t_out[:], t_mem[:], t_s[:, 0:1], grp_psum[g][:],
                        op0=AL.mult, op1=AL.add)
                    del grp_psum[g]
                else:
                    nc.vector.tensor_scalar_mul(t_out[:], t_mem[:], t_s[:, 0:1])
                nc.sync.dma_start(out_n[g * 128:(g + 1) * 128, :], t_out[:])

            grp_psum = {}
            upd_n = {}
            upd_r = {}

            def node_proj(ec):
                t_en = spool.tile([128, KT, 128], F16, tag="er", name=f"en_{ec}")
                nc.sync.dma_start(t_en[:], emb_n[ec].rearrange("p (k j) -> p k j", k=KT))
                p_u = pupool.tile([128, MEM_DIM], F32, tag="pu", name=f"pun_{ec}")
                for k in range(KT):
                    nc.tensor.matmul(p_u[:], t_en[:, k, :], t_wn[:, k, :],
                                     start=(k == 0), stop=(k == KT - 1))
                ohs = []
                for g in spans_n[ec]:
                    t_oh = ohpool.tile([128, 128], F32R, tag="oh", name=f"ohn_{ec}_{g}")
                    nc.vector.tensor_scalar(
                        t_oh[:], t_iota[:], float(g * 128), t_ids_n[:, ec:ec + 1],
                        op0=AL.add, op1=AL.is_equal)
                    ohs.append(t_oh)
                t_upd = updpool.tile([128, MEM_DIM], F32R, tag="upd", name=f"updn_{ec}")
                nc.vector.tensor_tensor(t_upd[:], p_u[:], t_bn[:], op=AL.add)
                upd_n[ec] = (t_upd, ohs)

            def node_scatter(ec):
                t_upd, ohs = upd_n.pop(ec)
                for t_oh, g in zip(ohs, spans_n[ec]):
                    if g not in grp_psum:
                        grp_psum[g] = pgpool.tile([128, MEM_DIM], F32, tag="pg",
                                                  name=f"pg_{g}")
                        first = True
                    else:
                        first = False
                    nc.tensor.matmul(grp_psum[g][:], t_oh[:], t_upd[:],
                                     start=first, stop=(last_chunk[g] == ec),
                                     skip_group_check=True)
                for g in sorted(merge_after[ec]):
                    merge_group(g)

            def rel_proj(ec):
                t_er2 = spool.tile([128, KT, 128], F16, tag="er", name=f"er_{ec}")
                nc.sync.dma_start(t_er2[:], emb_r[ec].rearrange("p (k j) -> p k j", k=KT))
                p_u = pupool.tile([128, MEM_DIM], F32, tag="pu", name=f"pur_{ec}")
                for k in range(KT):
                    nc.tensor.matmul(p_u[:], t_er2[:, k, :], t_wr[:, k, :],
                                     start=(k == 0), stop=(k == KT - 1))
                t_oh = ohpool.tile([128, 128], F32R, tag="oh", name=f"ohr_{ec}")
                nc.vector.tensor_scalar(
                    t_oh[:], t_iota[:], 0.0, t_ids_r[:, ec:ec + 1],
                    op0=AL.add, op1=AL.is_equal)
                t_upd = updpool.tile([128, MEM_DIM], F32R, tag="upd", name=f"updr_{ec}")
                nc.vector.tensor_tensor(t_upd[:], p_u[:], t_br[:], op=AL.add)
                upd_r[ec] = (t_upd, t_oh)

            def rel_scatter(ec):
                t_upd, t_oh = upd_r.pop(ec)
                nc.tensor.matmul(p_rel[:64, :], t_oh[:, :64], t_upd[:],
                                 start=(ec == 0), stop=(ec == NCr - 1),
                                 skip_group_check=True)

            # software-pipelined emission: scatter runs one chunk behind proj,
            # node/rel interleaved to smooth the engine mix
            p_rel = prpool.tile([128, MEM_DIM], F32, tag="prel")
            steps = []
            for i in range(max(NCn, NCr)):
                if i < NCn:
                    steps.append(("n", i))
                if i < NCr:
                    steps.append(("r", i))
            for j, (kind, i) in enumerate(steps):
                (node_proj if kind == "n" else rel_proj)(i)
                if j >= 1:
                    pk, pi = steps[j - 1]
                    (node_scatter if pk == "n" else rel_scatter)(pi)
            lk, li = steps[-1]
            (node_scatter if lk == "n" else rel_scatter)(li)

            for g in untouched:
                merge_group(g)

            # ---- rel merge ----
            t_rmem = wpool.tile([128, MEM_DIM], F32, tag="memst")
            nc.sync.dma_start(t_rmem[:64, :], rmem[:])
            t_rout = wpool.tile([128, MEM_DIM], F32, tag="outsb")
            nc.vector.scalar_tensor_tensor(
                t_rout[:64, :], t_rmem[:64, :], t_s[:64, 0:1], p_rel[:64, :],
                op0=AL.mult, op1=AL.add)
            nc.sync.dma_start(out_r[:], t_rout[:64, :])

    nc.finalize()
    return nc


def _route(ids, n_rows_per_core, pad_chunks=1):
    """Route events to owner cores; sort by local id.

    Returns (perm[core] event indices sorted by local id, NC common chunk count).
    """
    owner = np.minimum(ids // n_rows_per_core, NCORES - 1)
    perms = []
    for c in range(NCORES):
        ev = np.nonzero(owner == c)[0]
        loc = ids[ev] - c * n_rows_per_core
        order = np.argsort(loc, kind="stable")
        perms.append(ev[order])
    nmax = max(len(p) for p in perms)
    NC = (nmax + 127) // 128
    return perms, max(NC, 1)


def _rnd_f32r(x):
    b = x.view(np.uint32)
    low = b & np.uint32(0xFFF)
    keep = b & ~np.uint32(0xFFF)
    rup = keep + np.uint32(0x1000)
    use_up = (low > 0x800) | ((low == 0x800) & (((b >> 12) & 1) == 1))
    return np.where(use_up, rup, keep).view(np.float32)


def _pack_emb(embT, perm, NC):
    """embT [IN_DIM, B] fp32 -> [NC, 128, KT*128] routed/padded/pretiled."""
    n = len(perm)
    C = NC * 128
    # gather columns -> [IN_DIM, C]
    g = np.zeros((IN_DIM, C), dtype=embT.dtype)
    g[:, :n] = embT[:, perm]
    # [KT,128,NC,128] -> [NC, p=128(kdim), KT, 128(event)]
    g = g.reshape(KT, 128, NC, 128).transpose(2, 1, 0, 3).reshape(NC, 128, KT * 128)
    return np.ascontiguousarray(g)


def _pack_ids(local_ids, NC):
    n = len(local_ids)
    C = NC * 128
    out = np.full(C, PAD_ID, dtype=np.float32)
    out[:n] = local_ids.astype(np.float32)
    return np.ascontiguousarray(out.reshape(NC, 128).T)  # [128, NC]


def _spans(local_sorted_per_core, NC):
    spans = [set() for _ in range(NC)]
    for loc in local_sorted_per_core:
        for ec in range(NC):
            seg = loc[ec * 128:(ec + 1) * 128]
            if len(seg) == 0:
                continue
            for g in range(int(seg[0]) // 128, int(seg[-1]) // 128 + 1):
                spans[ec].add(g)
    return [sorted(s) for s in spans]


def kernel(nodes_embeddings, rels_embeddings, nodes_ids, rels_ids,
           entity_memory, rel_memory, W_node, b_node, W_rel, b_rel, time):
    nodes_embeddings = np.ascontiguousarray(np.asarray(nodes_embeddings, dtype=np.float32))
    rels_embeddings = np.ascontiguousarray(np.asarray(rels_embeddings, dtype=np.float32))
    nodes_ids = np.asarray(nodes_ids).astype(np.int64)
    rels_ids = np.asarray(rels_ids).astype(np.int64)
    entity_memory = np.asarray(entity_memory, dtype=np.float32)
    rel_memory = np.asarray(rel_memory, dtype=np.float32)
    W_node = np.asarray(W_node, dtype=np.float32)
    b_node = np.asarray(b_node, dtype=np.float32)
    W_rel = np.asarray(W_rel, dtype=np.float32)
    b_rel = np.asarray(b_rel, dtype=np.float32)
    t = float(np.asarray(time))

    inv = np.float32(1.0 / (t + 1.0))
    scale = np.float32(t / (t + 1.0)) if t > 1 else np.float32(1.0)

    # ---- host routing ----
    perms_n, NCn = _route(nodes_ids, NSHARD)
    perms_r, NCr = _route(rels_ids, RSHARD)

    loc_n = [nodes_ids[p] - c * NSHARD for c, p in enumerate(perms_n)]
    spans_n = _spans(loc_n, NCn)

    key = (NCn, NCr, tuple(tuple(s) for s in spans_n))
    if key not in _module_cache:
        _module_cache[key] = _build_module(NCn, NCr, spans_n)
    nc = _module_cache[key]

    # ---- host packing ----
    embT_n = nodes_embeddings.astype(np.float16).T  # [IN_DIM, B]
    embT_r = rels_embeddings.astype(np.float16).T
    wn = np.ascontiguousarray(
        (W_node * inv).T.reshape(KT, 128, MEM_DIM).transpose(1, 0, 2)
        .reshape(128, KT * MEM_DIM)).astype(np.float16)
    wr = np.ascontiguousarray(
        (W_rel * inv).T.reshape(KT, 128, MEM_DIM).transpose(1, 0, 2)
        .reshape(128, KT * MEM_DIM)).astype(np.float16)
    bn = np.broadcast_to(b_node * inv, (128, MEM_DIM)).astype(np.float32).copy()
    br = np.broadcast_to(b_rel * inv, (128, MEM_DIM)).astype(np.float32).copy()
    s_col = np.full((128, 1), scale, dtype=np.float32)
    iota = np.broadcast_to(np.arange(128, dtype=np.float32), (128, 128)).copy()

    in_maps = []
    for c in range(NCORES):
        lo_n, hi_n = c * NSHARD, min((c + 1) * NSHARD, N_NODES)
        lo_r, hi_r = c * RSHARD, min((c + 1) * RSHARD, N_RELS)
        mem_shard = np.zeros((NSHARD, MEM_DIM), dtype=np.float32)
        mem_shard[:hi_n - lo_n] = entity_memory[lo_n:hi_n]
        rmem_shard = np.zeros((RSHARD, MEM_DIM), dtype=np.float32)
        rmem_shard[:hi_r - lo_r] = rel_memory[lo_r:hi_r]
        in_maps.append(dict(
            emb_n=_pack_emb(embT_n, perms_n[c], NCn),
            emb_r=_pack_emb(embT_r, perms_r[c], NCr),
            ids_n=_pack_ids(loc_n[c], NCn),
            ids_r=_pack_ids(rels_ids[perms_r[c]] - c * RSHARD, NCr),
            w_n=wn, w_r=wr, b_n=bn, b_r=br, s_col=s_col, iota_in=iota,
            mem=mem_shard, rmem=rmem_shard,
        ))

    trace = bool(int(os.environ.get("KERNEL_TRACE", "0"))) and _ensure_ntff_hook()
    res = run_bass_kernel_spmd(
        nc, in_maps, core_ids=list(range(NCORES)),
        trace=trace, trace_cores=list(range(NCORES)) if trace else None)
    kernel.last_exec_time_ns = res.exec_time_ns
    kernel.last_results = res

    out = np.empty((N_NODES + N_RELS, MEM_DIM), dtype=np.float32)
    for c in range(NCORES):
        lo_n, hi_n = c * NSHARD, min((c + 1) * NSHARD, N_NODES)
        out[lo_n:hi_n] = res.results[c]["out_n"][:hi_n - lo_n]
        lo_r, hi_r = c * RSHARD, min((c + 1) * RSHARD, N_RELS)
        out[N_NODES + lo_r:N_NODES + hi_r] = res.results[c]["out_r"][:hi_r - lo_r]
    return out


# revision 10
# speedup vs baseline: 1.2299x; 1.0105x over previous
"""Trainium2 Bass kernel for nn_Memory scatter_memory problem.

Reference computation:
    scale = t/(t+1) if t > 1 else 1
    inv   = 1/(t+1)
    entity_memory = entity_memory*scale ;  .at[nodes_ids].add((nodes_emb @ W_node.T + b_node)*inv)
    rel_memory    = rel_memory*scale    ;  .at[rels_ids].add((rels_emb @ W_rel.T + b_rel)*inv)
    out = concat([entity_memory, rel_memory])   # [100500, 512]

Strategy (8 NeuronCores, SPMD single program):
  - Row-shard entity_memory (12544 rows/core) and rel_memory (64 rows/core).
  - HOST routes each event to its owner core (by id range), sorts by local row id,
    pads to a common chunk count, pre-transposes embeddings to [ev-chunk, p=kdim, 8, 128]
    layout, and pre-scales W/b by inv (so device updates are final).
  - DEVICE per core: fp32r projection matmuls (events on PSUM partitions), then
    scatter-add via one-hot matmuls into per-row-group PSUM tiles (exact fp32
    accumulation, duplicates handled by matmul), then out = mem*scale + psum.
  - Host reassembles the full [100500, 512] output from per-core shards.
"""

import os
import sys
import numpy as np

for _p in ("/root/.axon_site", "/root/.axon_site/_ro/trn_rl_repo",
           "/root/.axon_site/_ro/pypackages", "/opt/trn_rl_repo"):
    if os.path.isdir(_p) and _p not in sys.path:
        sys.path.append(_p)

import concourse.bacc as bacc
import concourse.mybir as mybir
import concourse.tile as tile
from concourse.bass_utils import run_bass_kernel_spmd

F32 = mybir.dt.float32
F32R = mybir.dt.float32r
F16 = mybir.dt.float16
AL = mybir.AluOpType

N_NODES = 100000
N_RELS = 500
MEM_DIM = 512
IN_DIM = 1024
NCORES = 8
NSHARD = 12544          # 98 * 128 rows per core (core 7 ragged, padded)
NGROUPS = NSHARD // 128  # 98
RSHARD = 64             # rel rows per core (core 7 ragged, padded)
KT = IN_DIM // 128      # 8 k-tiles
PAD_ID = 1.0e6

_module_cache = {}


def _ensure_ntff_hook():
    """Register the axon NTFF profile hook (missing antenv.axon_hooks shim)."""
    import types
    try:
        from antenv.axon_hooks import get_axon_ntff_profile_hook
        return get_axon_ntff_profile_hook() is not None
    except ImportError:
        pass
    try:
        import antenv
        from trn_agent_boot.trn_boot import _ntff_profile_via_ctypes
        import concourse.bass_utils as bu
        mod = types.ModuleType("antenv.axon_hooks")
        state = {"h": None}
        mod.set_axon_ntff_profile_hook = lambda h: state.__setitem__("h", h)
        mod.get_axon_ntff_profile_hook = lambda: state["h"]
        sys.modules["antenv.axon_hooks"] = mod
        antenv.axon_hooks = mod
        h = _ntff_profile_via_ctypes("/opt/axon/libaxon_pjrt.so")
        mod.set_axon_ntff_profile_hook(h)
        bu.upload_artifacts = lambda tmpdir: f"local:{tmpdir}"
        return h is not None
    except Exception:
        return False


def _build_module(NCn, NCr, spans_n):
    """Build the SPMD Bacc module.

    NCn/NCr: number of 128-event chunks for nodes/rels.
    spans_n: list over ec of sorted group lists (union over cores).
    """
    nc = bacc.Bacc(None, target_bir_lowering=False)

    emb_n = nc.dram_tensor("emb_n", [NCn, 128, KT * 128], F16, kind="ExternalInput")
    emb_r = nc.dram_tensor("emb_r", [NCr, 128, KT * 128], F16, kind="ExternalInput")
    ids_n = nc.dram_tensor("ids_n", [128, NCn], F32, kind="ExternalInput")
    ids_r = nc.dram_tensor("ids_r", [128, NCr], F32, kind="ExternalInput")
    w_n = nc.dram_tensor("w_n", [128, KT * MEM_DIM], F16, kind="ExternalInput")
    w_r = nc.dram_tensor("w_r", [128, KT * MEM_DIM], F16, kind="ExternalInput")
    b_n = nc.dram_tensor("b_n", [128, MEM_DIM], F32, kind="ExternalInput")
    b_r = nc.dram_tensor("b_r", [128, MEM_DIM], F32, kind="ExternalInput")
    s_col = nc.dram_tensor("s_col", [128, 1], F32, kind="ExternalInput")
    iota_in = nc.dram_tensor("iota_in", [128, 128], F32, kind="ExternalInput")
    mem = nc.dram_tensor("mem", [NSHARD, MEM_DIM], F32, kind="ExternalInput")
    rmem = nc.dram_tensor("rmem", [RSHARD, MEM_DIM], F32, kind="ExternalInput")
    out_n = nc.dram_tensor("out_n", [NSHARD, MEM_DIM], F32, kind="ExternalOutput")
    out_r = nc.dram_tensor("out_r", [RSHARD, MEM_DIM], F32, kind="ExternalOutput")

    # which chunk finishes each node group (merge point)
    last_chunk = {}
    for ec, gs in enumerate(spans_n):
        for g in gs:
            last_chunk[g] = ec
    merge_after = [[] for _ in range(NCn)]
    for g, ec in last_chunk.items():
        merge_after[ec].append(g)
    untouched = [g for g in range(NGROUPS) if g not in last_chunk]

    # PSUM budget: proj double-buffer + open scatter groups + rel accumulator
    maxopen = 0
    open_now = set()
    for ec, gs in enumerate(spans_n):
        open_now.update(gs)
        maxopen = max(maxopen, len(open_now))
        for g in merge_after[ec]:
            open_now.discard(g)
    pu_bufs = 2 if maxopen <= 5 else 1
    pg_bufs = min(max(maxopen, 1), 8 - pu_bufs - 1)

    with tile.TileContext(nc) as tc:
        with tc.tile_pool(name="const", bufs=1) as cpool, \
             tc.tile_pool(name="stage", bufs=10) as spool, \
             tc.tile_pool(name="work", bufs=14) as wpool, \
             tc.tile_pool(name="oh", bufs=16) as ohpool, \
             tc.tile_pool(name="updp", bufs=8) as updpool, \
             tc.tile_pool(name="pu", bufs=pu_bufs, space="PSUM") as pupool, \
             tc.tile_pool(name="pg", bufs=pg_bufs, space="PSUM") as pgpool, \
             tc.tile_pool(name="pr", bufs=1, space="PSUM") as prpool:

            # ---- constants (W first: PE-critical path) ----
            t_wn = cpool.tile([128, KT, MEM_DIM], F16, tag="wn")
            nc.sync.dma_start(t_wn[:], w_n.ap().rearrange("p (k n) -> p k n", k=KT))
            t_wr = cpool.tile([128, KT, MEM_DIM], F16, tag="wr")
            nc.sync.dma_start(t_wr[:], w_r.ap().rearrange("p (k n) -> p k n", k=KT))
            t_iota = cpool.tile([128, 128], F32, tag="iota")
            nc.scalar.dma_start(t_iota[:], iota_in[:])
            t_ids_n = cpool.tile([128, NCn], F32, tag="idsn")
            nc.scalar.dma_start(t_ids_n[:], ids_n[:])
            t_ids_r = cpool.tile([128, NCr], F32, tag="idsr")
            nc.scalar.dma_start(t_ids_r[:], ids_r[:])
            t_bn = cpool.tile([128, MEM_DIM], F32, tag="bn")
            nc.scalar.dma_start(t_bn[:], b_n[:])
            t_br = cpool.tile([128, MEM_DIM], F32, tag="br")
            nc.scalar.dma_start(t_br[:], b_r[:])
            t_s = cpool.tile([128, 1], F32, tag="scol")
            nc.scalar.dma_start(t_s[:], s_col[:])

            def merge_group(g):
                t_mem = wpool.tile([128, MEM_DIM], F32, tag="memst")
                nc.sync.dma_start(t_mem[:], mem[g * 128:(g + 1) * 128, :])
                t_out = wpool.tile([128, MEM_DIM], F32, tag="outsb")
                if g in grp_psum:
                    nc.vector.scalar_tensor_tensor(
                        t_out[:], t_mem[:], t_s[:, 0:1], grp_psum[g][:],
                        op0=AL.mult, op1=AL.add)
                    del grp_psum[g]
                else:
                    nc.vector.tensor_scalar_mul(t_out[:], t_mem[:], t_s[:, 0:1])
                nc.sync.dma_start(out_n[g * 128:(g + 1) * 128, :], t_out[:])

            grp_psum = {}
            upd_n = {}
            upd_r = {}

            def node_proj(ec):
                t_en = spool.tile([128, KT, 128], F16, tag="er", name=f"en_{ec}")
                nc.sync.dma_start(t_en[:], emb_n[ec].rearrange("p (k j) -> p k j", k=KT))
                p_u = pupool.tile([128, MEM_DIM], F32, tag="pu", name=f"pun_{ec}")
                for k in range(KT):
                    nc.tensor.matmul(p_u[:], t_en[:, k, :], t_wn[:, k, :],
                                     start=(k == 0), stop=(k == KT - 1))
                ohs = []
                for g in spans_n[ec]:
                    t_oh = ohpool.tile([128, 128], F32R, tag="oh", name=f"ohn_{ec}_{g}")
                    nc.vector.tensor_scalar(
                        t_oh[:], t_iota[:], float(g * 128), t_ids_n[:, ec:ec + 1],
                        op0=AL.add, op1=AL.is_equal)
                    ohs.append(t_oh)
                t_upd = updpool.tile([128, MEM_DIM], F32R, tag="upd", name=f"updn_{ec}")
                nc.vector.tensor_tensor(t_upd[:], p_u[:], t_bn[:], op=AL.add)
                upd_n[ec] = (t_upd, ohs)

            def node_scatter(ec):
                t_upd, ohs = upd_n.pop(ec)
                for t_oh, g in zip(ohs, spans_n[ec]):
                    if g not in grp_psum:
                        grp_psum[g] = pgpool.tile([128, MEM_DIM], F32, tag="pg",
                                                  name=f"pg_{g}")
                        first = True
                    else:
                        first = False
                    nc.tensor.matmul(grp_psum[g][:], t_oh[:], t_upd[:],
                                     start=first, stop=(last_chunk[g] == ec),
                                     skip_group_check=True)
                for g in sorted(merge_after[ec]):
                    merge_group(g)

            def rel_proj(ec):
                t_er2 = spool.tile([128, KT, 128], F16, tag="er", name=f"er_{ec}")
                nc.sync.dma_start(t_er2[:], emb_r[ec].rearrange("p (k j) -> p k j", k=KT))
                p_u = pupool.tile([128, MEM_DIM], F32, tag="pu", name=f"pur_{ec}")
                for k in range(KT):
                    nc.tensor.matmul(p_u[:], t_er2[:, k, :], t_wr[:, k, :],
                                     start=(k == 0), stop=(k == KT - 1))
                t_oh = ohpool.tile([128, 128], F32R, tag="oh", name=f"ohr_{ec}")
                nc.vector.tensor_scalar(
                    t_oh[:], t_iota[:], 0.0, t_ids_r[:, ec:ec + 1],
                    op0=AL.add, op1=AL.is_equal)
                t_upd = updpool.tile([128, MEM_DIM], F32R, tag="upd", name=f"updr_{ec}")
                nc.vector.tensor_tensor(t_upd[:], p_u[:], t_br[:], op=AL.add)
                upd_r[ec] = (t_upd, t_oh)

            def rel_scatter(ec):
                t_upd, t_oh = upd_r.pop(ec)
                nc.tensor.matmul(p_rel[:64, :], t_oh[:, :64], t_upd[:],
                                 start=(ec == 0), stop=(ec == NCr - 1),
                                 skip_group_check=True)

            # software-pipelined emission: scatter runs one chunk behind proj,
            # node/rel interleaved to smooth the engine mix
            p_rel = prpool.tile([128, MEM_DIM], F32, tag="prel")
            steps = []
            for i in range(max(NCn, NCr)):
                if i < NCn:
                    steps.append(("n", i))
                if i < NCr:
                    steps.append(("r", i))
            LAG = 2
            for j, (kind, i) in enumerate(steps):
                (node_proj if kind == "n" else rel_proj)(i)
                if j >= LAG:
                    pk, pi = steps[j - LAG]
                    (node_scatter if pk == "n" else rel_scatter)(pi)
            for j in range(max(len(steps) - LAG, 0), len(steps)):
                lk, li = steps[j]
                (node_scatter if lk == "n" else rel_scatter)(li)

            for g in untouched:
                merge_group(g)

            # ---- rel merge ----
            t_rmem = wpool.tile([128, MEM_DIM], F32, tag="memst")
            nc.sync.dma_start(t_rmem[:64, :], rmem[:])
            t_rout = wpool.tile([128, MEM_DIM], F32, tag="outsb")
            nc.vector.scalar_tensor_tensor(
                t_rout[:64, :], t_rmem[:64, :], t_s[:64, 0:1], p_rel[:64, :],
                op0=AL.mult, op1=AL.add)
            nc.sync.dma_start(out_r[:], t_rout[:64, :])

    nc.finalize()
    return nc


def _route(ids, n_rows_per_core, pad_chunks=1):
    """Route events to owner cores; sort by local id.

    Returns (perm[core] event indices sorted by local id, NC common chunk count).
    """
    owner = np.minimum(ids // n_rows_per_core, NCORES - 1)
    perms = []
    for c in range(NCORES):
        ev = np.nonzero(owner == c)[0]
        loc = ids[ev] - c * n_rows_per_core
        order = np.argsort(loc, kind="stable")
        perms.append(ev[order])
    nmax = max(len(p) for p in perms)
    NC = (nmax + 127) // 128
    return perms, max(NC, 1)


def _rnd_f32r(x):
    b = x.view(np.uint32)
    low = b & np.uint32(0xFFF)
    keep = b & ~np.uint32(0xFFF)
    rup = keep + np.uint32(0x1000)
    use_up = (low > 0x800) | ((low == 0x800) & (((b >> 12) & 1) == 1))
    return np.where(use_up, rup, keep).view(np.float32)


def _pack_emb(embT, perm, NC):
    """embT [IN_DIM, B] fp32 -> [NC, 128, KT*128] routed/padded/pretiled."""
    n = len(perm)
    C = NC * 128
    # gather columns -> [IN_DIM, C]
    g = np.zeros((IN_DIM, C), dtype=embT.dtype)
    g[:, :n] = embT[:, perm]
    # [KT,128,NC,128] -> [NC, p=128(kdim), KT, 128(event)]
    g = g.reshape(KT, 128, NC, 128).transpose(2, 1, 0, 3).reshape(NC, 128, KT * 128)
    return np.ascontiguousarray(g)


def _pack_ids(local_ids, NC):
    n = len(local_ids)
    C = NC * 128
    out = np.full(C, PAD_ID, dtype=np.float32)
    out[:n] = local_ids.astype(np.float32)
    return np.ascontiguousarray(out.reshape(NC, 128).T)  # [128, NC]


def _spans(local_sorted_per_core, NC):
    spans = [set() for _ in range(NC)]
    for loc in local_sorted_per_core:
        for ec in range(NC):
            seg = loc[ec * 128:(ec + 1) * 128]
            if len(seg) == 0:
                continue
            for g in range(int(seg[0]) // 128, int(seg[-1]) // 128 + 1):
                spans[ec].add(g)
    return [sorted(s) for s in spans]


def kernel(nodes_embeddings, rels_embeddings, nodes_ids, rels_ids,
           entity_memory, rel_memory, W_node, b_node, W_rel, b_rel, time):
    nodes_embeddings = np.ascontiguousarray(np.asarray(nodes_embeddings, dtype=np.float32))
    rels_embeddings = np.ascontiguousarray(np.asarray(rels_embeddings, dtype=np.float32))
    nodes_ids = np.asarray(nodes_ids).astype(np.int64)
    rels_ids = np.asarray(rels_ids).astype(np.int64)
    entity_memory = np.asarray(entity_memory, dtype=np.float32)
    rel_memory = np.asarray(rel_memory, dtype=np.float32)
    W_node = np.asarray(W_node, dtype=np.float32)
    b_node = np.asarray(b_node, dtype=np.float32)
    W_rel = np.asarray(W_rel, dtype=np.float32)
    b_rel = np.asarray(b_rel, dtype=np.float32)
    t = float(np.asarray(time))

    inv = np.float32(1.0 / (t + 1.0))
    scale = np.float32(t / (t + 1.0)) if t > 1 else np.float32(1.0)

    # ---- host routing ----
    perms_n, NCn = _route(nodes_ids, NSHARD)
    perms_r, NCr = _route(rels_ids, RSHARD)

    loc_n = [nodes_ids[p] - c * NSHARD for c, p in enumerate(perms_n)]
    spans_n = _spans(loc_n, NCn)

    key = (NCn, NCr, tuple(tuple(s) for s in spans_n))
    if key not in _module_cache:
        _module_cache[key] = _build_module(NCn, NCr, spans_n)
    nc = _module_cache[key]

    # ---- host packing ----
    embT_n = nodes_embeddings.astype(np.float16).T  # [IN_DIM, B]
    embT_r = rels_embeddings.astype(np.float16).T
    wn = np.ascontiguousarray(
        (W_node * inv).T.reshape(KT, 128, MEM_DIM).transpose(1, 0, 2)
        .reshape(128, KT * MEM_DIM)).astype(np.float16)
    wr = np.ascontiguousarray(
        (W_rel * inv).T.reshape(KT, 128, MEM_DIM).transpose(1, 0, 2)
        .reshape(128, KT * MEM_DIM)).astype(np.float16)
    bn = np.broadcast_to(b_node * inv, (128, MEM_DIM)).astype(np.float32).copy()
    br = np.broadcast_to(b_rel * inv, (128, MEM_DIM)).astype(np.float32).copy()
    s_col = np.full((128, 1), scale, dtype=np.float32)
    iota = np.broadcast_to(np.arange(128, dtype=np.float32), (128, 128)).copy()

    in_maps = []
    for c in range(NCORES):
        lo_n, hi_n = c * NSHARD, min((c + 1) * NSHARD, N_NODES)
        lo_r, hi_r = c * RSHARD, min((c + 1) * RSHARD, N_RELS)
        mem_shard = np.zeros((NSHARD, MEM_DIM), dtype=np.float32)
        mem_shard[:hi_n - lo_n] = entity_memory[lo_n:hi_n]
        rmem_shard = np.zeros((RSHARD, MEM_DIM), dtype=np.float32)
        rmem_shard[:hi_r - lo_r] = rel_memory[lo_r:hi_r]
        in_maps.append(dict(
            emb_n=_pack_emb(embT_n, perms_n[c], NCn),
            emb_r=_pack_emb(embT_r, perms_r[c], NCr),
            ids_n=_pack_ids(loc_n[c], NCn),
            ids_r=_pack_ids(rels_ids[perms_r[c]] - c * RSHARD, NCr),
            w_n=wn, w_r=wr, b_n=bn, b_r=br, s_col=s_col, iota_in=iota,
            mem=mem_shard, rmem=rmem_shard,
        ))

    trace = bool(int(os.environ.get("KERNEL_TRACE", "0"))) and _ensure_ntff_hook()
    res = run_bass_kernel_spmd(
        nc, in_maps, core_ids=list(range(NCORES)),
        trace=trace, trace_cores=list(range(NCORES)) if trace else None)
    kernel.last_exec_time_ns = res.exec_time_ns
    kernel.last_results = res

    out = np.empty((N_NODES + N_RELS, MEM_DIM), dtype=np.float32)
    for c in range(NCORES):
        lo_n, hi_n = c * NSHARD, min((c + 1) * NSHARD, N_NODES)
        out[lo_n:hi_n] = res.results[c]["out_n"][:hi_n - lo_n]
        lo_r, hi_r = c * RSHARD, min((c + 1) * RSHARD, N_RELS)
        out[N_NODES + lo_r:N_NODES + hi_r] = res.results[c]["out_r"][:hi_r - lo_r]
    return out
